# revision 1
# baseline (speedup 1.0000x reference)
"""Chamfer distance loss kernel for Trainium2 (8 NeuronCores).

Problem: points1, points2 [8, 4096, 3] fp32 -> scalar loss.
Sharding: data-parallel over batch; core b handles batch b. Host averages the
8 per-batch losses.

Per-core algorithm:
  dist[i,j] = n1[i] + n2[j] - 2*x1[i].x2[j]  (squared L2)
  * TensorE: PSUM[i,j] = sum_k L[k,i]*R[k,j] where the 21 live rows are a
    3-level bf16 split of the coordinates (hi/lo/lo2) plus rows carrying
    -n_j/2 (3-level bf16 split), so PSUM = (x_i.x_j)_fp32ish - n_j/2.
    The 21 operand rows (zero-padded to 32) are replicated at partition
    bases 0/32/64/96 and the PE is driven in 4x-row-tiling mode
    (tile_position=(32q, 0)): four K=21 matmuls execute concurrently in
    separate 32-row tiles, giving ~4x PE throughput at ~1/6 the MAC power
    of a K=128 matmul. The low power matters: sustained full-array
    matmuls on all 8 cores trip chip-level power throttling that slows
    every engine by 20-40%.
  * Reduction to -min_j dist[i,j], two engine paths balanced against each
    other (both directions of the chamfer loss use the same machinery with
    the clouds swapped, so everything is a free-dim reduction):
      path A (ScalarE-heavy): ACT Identity(psum*2 + (-n_i)) -> fp16 = -dist,
        then a VectorE fp16 max-tree (2x mode) + reduce_max.
      path B (VectorE-only): tensor_tensor_reduce straight off PSUM
        accumulates rowmax = max_j psum[i,j]; since -dist = 2*psum - n1 is
        monotonic in psum per row, a per-column fixup 2*rowmax - n1 gives
        the same result with no ACT work and no fp16 rounding.
  * Means: ones-vector matmul partition-sum of the per-point maxes, scaled
    by -1/4096.
"""

import numpy as np

N = 4096          # points per cloud
P = 128           # partitions
TT = N // P       # 32 column blocks
D3 = 3
JB = 512          # matmul moving free dim
HALF = N // 2     # per-PSUM-allocation j extent (4 banks)
B = 8             # batches / cores
KPAD = 128        # contraction dim padded so the PE HAM clock-gate stays warm
                  # (K>=96 streams at 2.4 GHz, K<=80 at 1.2; smaller K = less power)
N_PATH_B = 10     # of the 64 (direction, i-tile) units, how many use path B

_NC_CACHE = {}


def _build_nc():
    import concourse.bacc as bacc
    import concourse.tile as tile
    from concourse import mybir

    FP32 = mybir.dt.float32

    nc = bacc.Bacc("TRN2", target_bir_lowering=False, debug=False)
    p1 = nc.dram_tensor("points1", [N, D3], FP32, kind="ExternalInput").ap()
    p2 = nc.dram_tensor("points2", [N, D3], FP32, kind="ExternalInput").ap()
    ident_in = nc.dram_tensor("ident128", [P, P], FP32, kind="ExternalInput").ap()
    zeros_in = nc.dram_tensor("zeros4096", [1, N], mybir.dt.bfloat16,
                              kind="ExternalInput").ap()
    out = nc.dram_tensor("loss", [1, 1], FP32, kind="ExternalOutput").ap()

    with tile.TileContext(nc) as tc:
        _emit(tc, p1, p2, ident_in, zeros_in, out)

    nc.compile()
    return nc


def _emit(tc, p1, p2, ident_in, zeros_in, out):
    import concourse.bass as bass  # noqa: F401
    from concourse import mybir

    FP32 = mybir.dt.float32
    BF16 = mybir.dt.bfloat16
    FP16 = mybir.dt.float16
    AX = mybir.AxisListType
    OP = mybir.AluOpType
    AF = mybir.ActivationFunctionType

    nc = tc.nc

    # Row spec: pairs of (L-side source, R-side source) per coordinate.
    # H = bf16 hi, L = lo, L2 = lo2 of the raw coordinate values.
    COORD_PAIRS = [
        ("H", "H"), ("H", "L"), ("H", "L2"), ("L", "H"), ("L", "L"), ("L2", "H"),
    ]
    NROWS = len(COORD_PAIRS) * D3 + 3  # live rows; the rest are zero padding

    # Which (direction, i-tile) units take path B (VectorE-only reduction).
    n_units = 2 * TT
    # Spread path-B units evenly but end-aligned: the final units' reductions
    # are the pipeline drain, and path B has ~3us less post-matmul latency.
    path_b = {((k + 1) * n_units) // N_PATH_B - 1 for k in range(N_PATH_B)}

    from contextlib import ExitStack
    with ExitStack() as ctx:
        consts = ctx.enter_context(tc.tile_pool(name="consts", bufs=1))

        ident = consts.tile([P, P], FP32, name="ident", tag="ident")
        nc.sync.dma_start(out=ident, in_=ident_in)

        ones_col = consts.tile([P, 1], FP32, name="ones_col", tag="ones_col")
        nc.vector.memset(ones_col, 1.0)

        ones96 = consts.tile([TT * D3, P], BF16, name="ones96", tag="ones96")
        nc.vector.memset(ones96, 1.0)

        # Persistent per-direction operand buffers and biases.
        Lbufs, Rbufs, negNs = [], [], []
        for m in range(2):
            Lb = consts.tile([KPAD, N], BF16, name=f"Lbuf{m}", tag=f"Lbuf{m}")
            Rb = consts.tile([KPAD, N], BF16, name=f"Rbuf{m}", tag=f"Rbuf{m}")
            Lbufs.append(Lb)
            Rbufs.append(Rb)
            negNs.append(consts.tile([P, TT], FP32, name=f"negN{m}", tag=f"negN{m}"))
        RMAX = consts.tile([P, 2 * TT], FP32, name="RMAX", tag="RMAX")

        # ---------------- setup phase ----------------
        coord_srcs, norm_srcs = [], []
        with tc.tile_pool(name="pst", bufs=2, space="PSUM") as pst, \
             tc.tile_pool(name="stmp", bufs=1) as stmp:
            for m, X in enumerate((p1, p2)):
                S = stmp.tile([P, TT, D3], FP32, name=f"S{m}", tag=f"S{m}")
                nc.sync.dma_start(out=S, in_=X.rearrange("(p t) d -> p t d", p=P))

                SQ = stmp.tile([P, TT, D3], FP32, name=f"SQ{m}", tag=f"SQ{m}")
                nc.vector.tensor_mul(SQ, S, S)
                NP_ = stmp.tile([P, TT], FP32, name=f"NP{m}", tag=f"NP{m}")
                nc.vector.tensor_reduce(out=NP_, in_=SQ, axis=AX.X, op=OP.add)
                # ACT bias for the direction where this cloud is the i-side.
                nc.vector.tensor_scalar_mul(negNs[m], NP_, -1.0)

                # Transpose coords: S [128, 96] -> TS [96, 128] (fp32), with
                # coordinate d landing in the contiguous partition block
                # [32*d, 32*d+32). One transpose per coordinate because the
                # stationary matmul operand allows only one free dim.
                TS = stmp.tile([TT * D3, P], FP32, name=f"TS{m}", tag=f"TS{m}")
                for dd in range(D3):
                    in_d = S[:, :, dd:dd + 1].rearrange("p t e -> p (t e)")
                    tps = pst.tile([TT, P], FP32, name=f"tps{m}_{dd}", tag="tps")
                    nc.tensor.transpose(tps, in_d, ident)
                    nc.scalar.copy(TS[dd * TT:(dd + 1) * TT, :], tps)

                # 3-level bf16 split of coords.
                H = stmp.tile([TT * D3, P], BF16, name=f"H{m}", tag=f"H{m}")
                nc.vector.tensor_copy(H, TS)
                r1 = stmp.tile([TT * D3, P], FP32, name=f"r1_{m}", tag=f"r1_{m}")
                nc.vector.tensor_sub(r1, TS, H)
                Lo = stmp.tile([TT * D3, P], BF16, name=f"Lo{m}", tag=f"Lo{m}")
                nc.vector.tensor_copy(Lo, r1)
                r2 = stmp.tile([TT * D3, P], FP32, name=f"r2_{m}", tag=f"r2_{m}")
                nc.vector.tensor_sub(r2, r1, Lo)
                Lo2 = stmp.tile([TT * D3, P], BF16, name=f"Lo2{m}", tag=f"Lo2{m}")
                nc.vector.tensor_copy(Lo2, r2)

                # Norms transposed: NP [128, 32] -> [32, 128], scaled by -1/2,
                # then 3-level bf16 split.
                tpn = pst.tile([TT, P], FP32, name=f"tpn{m}", tag="tpn")
                nc.tensor.transpose(tpn, NP_, ident)
                TNn = stmp.tile([TT, P], FP32, name=f"TNn{m}", tag=f"TNn{m}")
                nc.scalar.mul(TNn, tpn, -0.5)
                NSPL = stmp.tile([TT * D3, P], BF16, name=f"NSPL{m}",
                                 tag=f"NSPL{m}")
                NH = stmp.tile([TT, P], BF16, name=f"NH{m}", tag=f"NH{m}")
                nc.vector.tensor_copy(NH, TNn)
                nr1 = stmp.tile([TT, P], FP32, name=f"nr1_{m}", tag=f"nr1_{m}")
                nc.vector.tensor_sub(nr1, TNn, NH)
                NL = stmp.tile([TT, P], BF16, name=f"NL{m}", tag=f"NL{m}")
                nc.vector.tensor_copy(NL, nr1)
                nr2 = stmp.tile([TT, P], FP32, name=f"nr2_{m}", tag=f"nr2_{m}")
                nc.vector.tensor_sub(nr2, nr1, NL)
                nc.vector.tensor_copy(NSPL[2 * TT:3 * TT, :], nr2)
                nc.scalar.copy(NSPL[0:TT, :], NH)
                nc.scalar.copy(NSPL[TT:2 * TT, :], NL)

                coord_srcs.append({"H": H, "L": Lo, "L2": Lo2})
                norm_srcs.append(NSPL)

            # Row assembly: column c = 128*t + p <-> point p*32 + t.
            # Pair-major row layout: rows [3q, 3q+3) hold pair q over coords
            # x,y,z, so each group is ONE dma from one full [96,128] source
            # tile (partition order 32*ci + t matches (ci, t) iteration).
            # The buffers direction 0 needs (Lbuf0, Rbuf1) are filled first,
            # on separate HWDGE queues, so the main loop starts earlier.
            def fill_rows(buf, m, side, eng):
                for q, pair in enumerate(COORD_PAIRS):
                    srct = coord_srcs[m][pair[0] if side == "L" else pair[1]]
                    dst = buf[3 * q:3 * q + 3, :].rearrange(
                        "r (t p) -> r t p", p=P)
                    eng.dma_start(out=dst, in_=srct)
                r0 = 3 * len(COORD_PAIRS)
                dst = buf[r0:r0 + 3, :].rearrange("r (t p) -> r t p", p=P)
                eng.dma_start(
                    out=dst, in_=ones96 if side == "L" else norm_srcs[m])

            def replicate(buf):
                # Copy the whole 32-row block (live rows + zeroed pad rows) to
                # partition bases 32/64/96 so the four 32-row PE tiles each
                # see the operands in their own quadrant. DVE bf16 copy runs
                # in 4x mode (~1.1us per copy).
                for q in (32, 64, 96):
                    nc.vector.tensor_copy(buf[q:q + 32, :], buf[0:32, :])

            def zero_pads(buf, eng):
                # Zero rows [NROWS, 32) of block 0 from DRAM (the replicate
                # copies then carry the zeros to the other three blocks).
                zsrc = bass.AP(tensor=zeros_in.tensor, offset=zeros_in.offset,
                               ap=[[0, 32 - NROWS], [1, N]])
                eng.dma_start(out=buf[NROWS:32, :], in_=zsrc)

            fill_rows(Lbufs[0], 0, "L", nc.scalar)
            fill_rows(Rbufs[1], 1, "R", nc.sync)
            zero_pads(Lbufs[0], nc.scalar)
            zero_pads(Rbufs[1], nc.sync)
            replicate(Lbufs[0])
            replicate(Rbufs[1])
            fill_rows(Lbufs[1], 1, "L", nc.scalar)
            fill_rows(Rbufs[0], 0, "R", nc.sync)
            zero_pads(Lbufs[1], nc.scalar)
            zero_pads(Rbufs[0], nc.sync)
            replicate(Lbufs[1])
            replicate(Rbufs[0])

        # ---------------- main loop ----------------
        unit = 0
        with tc.tile_pool(name="psm", bufs=2, space="PSUM") as psm, \
             tc.tile_pool(name="dpool", bufs=2) as dpool, \
             tc.tile_pool(name="papool", bufs=2) as papool, \
             tc.tile_pool(name="pbpool", bufs=2) as pbpool, \
             tc.tile_pool(name="scrpool", bufs=2) as scrpool:
            for d in range(2):
                Lb = Lbufs[0] if d == 0 else Lbufs[1]
                Rb = Rbufs[1] if d == 0 else Rbufs[0]
                bias = negNs[0] if d == 0 else negNs[1]
                for t in range(TT):
                    col = d * TT + t
                    use_b = unit in path_b
                    unit += 1
                    if use_b:
                        # --- path B: VectorE-only, straight off PSUM ---
                        rb2 = scrpool.tile([P, 2], FP32, name="rb2", tag="rb2")
                        for h in range(2):
                            ps = psm.tile([P, HALF], FP32, name="ps", tag="ps")
                            for u in range(HALF // JB):
                                j0 = h * HALF + u * JB
                                q = 32 * (u % 4)
                                nc.tensor.matmul(
                                    ps[:, u * JB:(u + 1) * JB],
                                    lhsT=Lb[q:q + NROWS, t * P:(t + 1) * P],
                                    rhs=Rb[q:q + NROWS, j0:j0 + JB],
                                    start=True, stop=True,
                                    tile_position=(q, 0),
                                )
                            nc.vector.tensor_reduce(
                                out=rb2[:, h:h + 1], in_=ps,
                                axis=AX.X, op=OP.max,
                            )
                        # fixup: -min dist = 2*max(psum) - n1
                        nc.vector.tensor_max(
                            rb2[:, 0:1], rb2[:, 0:1], rb2[:, 1:2])
                        nc.vector.scalar_tensor_tensor(
                            out=RMAX[:, col:col + 1],
                            in0=rb2[:, 0:1],
                            scalar=2.0,
                            in1=bias[:, t:t + 1],
                            op0=OP.mult,
                            op1=OP.add,
                        )
                    else:
                        # --- path A: ScalarE cast + VectorE fp16 max tree ---
                        Dt = dpool.tile([P, N], FP16, name="Dt", tag="Dt")
                        for h in range(2):
                            ps = psm.tile([P, HALF], FP32, name="ps", tag="ps")
                            for u in range(HALF // JB):
                                j0 = h * HALF + u * JB
                                q = 32 * (u % 4)
                                nc.tensor.matmul(
                                    ps[:, u * JB:(u + 1) * JB],
                                    lhsT=Lb[q:q + NROWS, t * P:(t + 1) * P],
                                    rhs=Rb[q:q + NROWS, j0:j0 + JB],
                                    start=True, stop=True,
                                    tile_position=(q, 0),
                                )
                            nc.scalar.activation(
                                out=Dt[:, h * HALF:(h + 1) * HALF],
                                in_=ps,
                                func=AF.Identity,
                                bias=bias[:, t:t + 1],
                                scale=2.0,
                            )
                        PA = papool.tile([P, HALF], FP16, name="PA", tag="PA")
                        PB = pbpool.tile([P, HALF // 2], FP16,
                                         name="PB", tag="PB")
                        nc.vector.tensor_max(PA, Dt[:, :HALF], Dt[:, HALF:])
                        nc.vector.tensor_max(PB, PA[:, :1024], PA[:, 1024:2048])
                        nc.vector.tensor_max(PA[:, :512], PB[:, :512],
                                             PB[:, 512:1024])
                        nc.vector.tensor_max(PB[:, :256], PA[:, :256],
                                             PA[:, 256:512])
                        nc.vector.tensor_reduce(
                            out=RMAX[:, col:col + 1],
                            in_=PB[:, :256], axis=AX.X, op=OP.max,
                        )

        # ---------------- final reduction ----------------
        with tc.tile_pool(name="psf", bufs=1, space="PSUM") as psf, \
             tc.tile_pool(name="ftmp", bufs=1) as ftmp:
            pss = psf.tile([1, 2 * TT], FP32, name="pss")
            nc.tensor.matmul(pss, lhsT=ones_col, rhs=RMAX, start=True, stop=True)
            ssum = ftmp.tile([1, 1], FP32, name="ssum", tag="ssum")
            nc.vector.tensor_reduce(out=ssum, in_=pss, axis=AX.X, op=OP.add)
            res = ftmp.tile([1, 1], FP32, name="res", tag="res")
            nc.vector.tensor_scalar_mul(res, ssum, -1.0 / N)
            nc.sync.dma_start(out=out, in_=res)


def get_nc():
    if "nc" not in _NC_CACHE:
        _NC_CACHE["nc"] = _build_nc()
    return _NC_CACHE["nc"]


def kernel(points1, points2, **_ignored):
    from concourse.bass_utils import run_bass_kernel_spmd

    p1 = np.ascontiguousarray(np.asarray(points1, dtype=np.float32))
    p2 = np.ascontiguousarray(np.asarray(points2, dtype=np.float32))
    assert p1.shape == (B, N, D3) and p2.shape == (B, N, D3)

    nc = get_nc()
    import ml_dtypes
    eye = np.eye(P, dtype=np.float32)
    zeros = np.zeros((1, N), dtype=ml_dtypes.bfloat16)
    in_maps = [
        {"points1": p1[b].reshape(N, D3), "points2": p2[b].reshape(N, D3),
         "ident128": eye, "zeros4096": zeros}
        for b in range(B)
    ]
    res = run_bass_kernel_spmd(nc, in_maps, core_ids=list(range(B)))
    losses = np.array(
        [res.results[b]["loss"][0, 0] for b in range(B)], dtype=np.float32
    )
    return np.float32(losses.mean())

